# revision 13
# baseline (speedup 1.0000x reference)
"""GQA causal attention block (B=4, S=1024, D=4096, H=32, KH=8, HD=128) on 8
Trainium2 NeuronCores.

Sharding v3: data-parallel over (batch, q-row-parity). Each batch b is split
between cores 2b (q-row blocks [7,5,3,1], "A") and 2b+1 (blocks [6,4,2,0],
"B"), 512 query tokens per core. Each core projects Q/K/V only for its OWN
512 tokens; the pair exchanges K and V via zero-padded pairwise AllReduces
(AllGather is broken in this runtime), so K/V projection work is not
duplicated. V is projected directly in transposed [tok, vdim] layout (x tile
as the stationary matmul operand), so no on-chip transposes are needed.

Causal load balance: q-row blocks are assigned so both cores of a pair need
the same per-slot key-prefix profile C=(8,6,4,2) (in 128-row k-tiles). The
attention loop iterates original k-tile index t=7..0 with a growing q-col
prefix (128,128,256,256,384,384,512,512 cols). The key tiles live in the
gathered buffer at fixed positions (traversal table); causality inside the
last 128-col block of each prefix is applied by multiplying the exp'd probs
with a per-core 0/1 mask tile (data), everything else uses a uniform -8 exp
bias. This computes 20/32 of the dense score tiles per head.

The output projection runs as two passes of 16 heads (the second overlaps
the attention tail); the two partial outputs are summed on the host.

RoPE trick (as baseline): wq/wk columns host-permuted per head to [even|odd]
halves; rot(q) = q*cos + (S@q)*sin with a constant 128x128 swap matrix.
cos/sin tables are precomputed on host from freqs_cis.

Matmuls in fp16; softmax statistics in fp32; exp biased by -8 (cancelled by
the normalization).
"""

import numpy as np

import concourse.bass as bass
import concourse.tile as tile
from concourse import bacc, mybir
from concourse.bass_utils import run_bass_kernel_spmd

B, S, D = 4, 1024, 4096
H, KH, HD = 32, 8, 128
HALF = S // 2                    # tokens per core
N_CORES = 8
SCALE = 1.0 / float(np.sqrt(HD))
EXP_BIAS = -8.0

MM_DT = mybir.dt.float16
MM_NP = np.float16
F32 = mybir.dt.float32

DT = D // 128                    # 32 d-tiles
GROUPS = [[0, 1], [2, 3], [4, 5], [6, 7]]

# q-row blocks per core type (slot order, need-descending)
BLOCKS_A = [7, 5, 3, 1]
BLOCKS_B = [6, 4, 2, 0]
# gathered position of original k-row t: A blocks at 0..3, B at 4..7
TRAV = [None] * 8
for _s, _r in enumerate(BLOCKS_A):
    TRAV[_r] = _s
for _s, _r in enumerate(BLOCKS_B):
    TRAV[_r] = 4 + _s
# q-col prefix width when processing original k-tile t
NCOLS = [512, 512, 384, 384, 256, 256, 128, 128]

_compiled = None


def _build():
    nc = bacc.Bacc("TRN2", target_bir_lowering=False, debug=False,
                   num_devices=N_CORES)

    xT = nc.dram_tensor("xT", [D, HALF], MM_DT, kind="ExternalInput").ap()
    wq = nc.dram_tensor("wq", [D, H * HD], MM_DT, kind="ExternalInput").ap()
    wk = nc.dram_tensor("wk", [D, KH * HD], MM_DT, kind="ExternalInput").ap()
    wv = nc.dram_tensor("wv", [D, KH * HD], MM_DT, kind="ExternalInput").ap()
    wo = nc.dram_tensor("wo", [H * HD, D], MM_DT, kind="ExternalInput").ap()
    cosT_d = nc.dram_tensor("cosT", [128, HALF], F32, kind="ExternalInput").ap()
    sinT_d = nc.dram_tensor("sinT", [128, HALF], F32, kind="ExternalInput").ap()
    dmask_d = nc.dram_tensor("dmask", [128, 8, 128], MM_DT, kind="ExternalInput").ap()
    mh0_d = nc.dram_tensor("mh0", [128, HALF], MM_DT, kind="ExternalInput").ap()
    mh1_d = nc.dram_tensor("mh1", [128, HALF], MM_DT, kind="ExternalInput").ap()
    rotT_d = nc.dram_tensor("rotT", [128, 128], MM_DT, kind="ExternalInput").ap()
    ones_d = nc.dram_tensor("ones", [128, 128], MM_DT, kind="ExternalInput").ap()
    out0 = nc.dram_tensor("out0", [HALF, D], F32, kind="ExternalOutput").ap()
    out1 = nc.dram_tensor("out1", [HALF, D], F32, kind="ExternalOutput").ap()

    # pair-exchange staging (K: [hd, tok] j-tiles; V: [tok, vd] tiles)
    stageK = nc.dram_tensor("stageK", [2, 8, 128, HALF], MM_DT).ap()
    gathK = nc.dram_tensor("gathK", [2, 8, 128, HALF], MM_DT).ap()
    stageV = nc.dram_tensor("stageV", [2, 8, 128, HALF], MM_DT).ap()
    gathV = nc.dram_tensor("gathV", [2, 8, 128, HALF], MM_DT).ap()

    from contextlib import ExitStack

    es = ExitStack()
    with tile.TileContext(nc) as tc, es:
        const = es.enter_context(tc.tile_pool(name="const", bufs=1))
        xbuf = es.enter_context(tc.tile_pool(name="xbuf", bufs=1))
        qbuf = es.enter_context(tc.tile_pool(name="qbuf", bufs=1))
        abuf = es.enter_context(tc.tile_pool(name="abuf", bufs=1))
        kbuf = es.enter_context(tc.tile_pool(name="kbuf", bufs=1))
        vbuf = es.enter_context(tc.tile_pool(name="vbuf", bufs=1))
        wbuf = es.enter_context(tc.tile_pool(name="wbuf", bufs=8))
        pw = es.enter_context(tc.tile_pool(name="pw", bufs=2))
        pr = es.enter_context(tc.tile_pool(name="pr", bufs=3))
        rb = es.enter_context(tc.tile_pool(name="rb", bufs=2))
        ow = es.enter_context(tc.tile_pool(name="ow", bufs=3))
        ps_proj = es.enter_context(tc.tile_pool(name="ps_proj", bufs=4, space="PSUM"))
        ps_attn = es.enter_context(tc.tile_pool(name="ps_attn", bufs=2, space="PSUM"))
        ps_sc = es.enter_context(tc.tile_pool(name="ps_sc", bufs=2, space="PSUM"))

        # ---- x first (scalar queue; weights go on sync in parallel) ----
        xT_s = xbuf.tile([128, DT, HALF], MM_DT, tag="x")

        def load_x(dc):
            nc.scalar.dma_start(
                out=xT_s[:, dc * 4:(dc + 1) * 4, :],
                in_=xT[dc * 512:(dc + 1) * 512, :].rearrange(
                    "(t p) c -> p t c", p=128))

        nc.scalar.dma_start(
            out=xT_s[:, 0:2, :],
            in_=xT[0:256, :].rearrange("(t p) c -> p t c", p=128))
        nc.scalar.dma_start(
            out=xT_s[:, 2:4, :],
            in_=xT[256:512, :].rearrange("(t p) c -> p t c", p=128))
        load_x(1)

        # ---- constants (scalar queue) ----
        rotT = const.tile([128, 128], MM_DT, tag="rot")
        nc.scalar.dma_start(out=rotT, in_=rotT_d)
        ones = const.tile([128, 128], MM_DT, tag="ones")
        nc.scalar.dma_start(out=ones, in_=ones_d)
        cosT = const.tile([128, HALF], F32, tag="cos")
        nc.scalar.dma_start(out=cosT, in_=cosT_d)
        sinT = const.tile([128, HALF], F32, tag="sin")
        nc.scalar.dma_start(out=sinT, in_=sinT_d)
        dmask = const.tile([128, 8, 128], MM_DT, tag="dmask")
        nc.scalar.dma_start(out=dmask, in_=dmask_d)
        mh0 = const.tile([128, HALF], MM_DT, tag="mh0")
        nc.scalar.dma_start(out=mh0, in_=mh0_d)
        mh1 = const.tile([128, HALF], MM_DT, tag="mh1")
        nc.scalar.dma_start(out=mh1, in_=mh1_d)
        m8 = const.tile([128, 1], F32, tag="m8")
        nc.vector.memset(m8, EXP_BIAS)
        for dc in range(2, 8):
            load_x(dc)

        def proj_group4(w_ap, jg):
            """Project 4 j-tiles (cols jg*512..+512 of w_ap) over all of D."""
            accs = [ps_proj.tile([128, HALF], F32, tag="acc", name=f"acc{i}")
                    for i in range(4)]
            for dp in range(8):
                w_t = wbuf.tile([128, 4, HALF], MM_DT, tag="w")
                nc.sync.dma_start(
                    out=w_t,
                    in_=w_ap[dp * 512:(dp + 1) * 512,
                             jg * 512:(jg + 1) * 512].rearrange(
                                 "(t p) j -> p t j", p=128))
                for t in range(4):
                    for jj in range(4):
                        nc.tensor.matmul(
                            accs[jj], w_t[:, t, jj * 128:(jj + 1) * 128],
                            xT_s[:, dp * 4 + t, :],
                            start=(dp == 0 and t == 0),
                            stop=(dp == 7 and t == 3))
            return accs

        def projT_group4(w_ap, vg):
            """Project 4 token-tiles of (x @ w[:, vg*512:+512]) TRANSPOSED:
            accs[t4] = [128 tok, 512 vdim], x tile as stationary operand."""
            accs = [ps_proj.tile([128, HALF], F32, tag="acc", name=f"vacc{i}")
                    for i in range(4)]
            for dp in range(8):
                w_t = wbuf.tile([128, 4, HALF], MM_DT, tag="w")
                nc.sync.dma_start(
                    out=w_t,
                    in_=w_ap[dp * 512:(dp + 1) * 512,
                             vg * 512:(vg + 1) * 512].rearrange(
                                 "(t p) j -> p t j", p=128))
                for t in range(4):
                    for t4 in range(4):
                        nc.tensor.matmul(
                            accs[t4],
                            xT_s[:, dp * 4 + t, t4 * 128:(t4 + 1) * 128],
                            w_t[:, t, :],
                            start=(dp == 0 and t == 0),
                            stop=(dp == 7 and t == 3))
            return accs

        def rope(acc):
            q_s = pw.tile([128, HALF], MM_DT, tag="qs")
            nc.scalar.copy(q_s, acc)
            ps2 = ps_proj.tile([128, HALF], F32, tag="acc", name="rot")
            nc.tensor.matmul(ps2, rotT, q_s, start=True, stop=True)
            t1 = pw.tile([128, HALF], F32, tag="t1")
            nc.vector.tensor_mul(t1, q_s, cosT)
            t2 = pw.tile([128, HALF], F32, tag="t2")
            nc.vector.tensor_mul(t2, ps2, sinT)
            return t1, t2

        def stage_pair(src, dst0, dst1):
            s0 = pw.tile([128, HALF], MM_DT, tag="ks0")
            nc.vector.tensor_mul(s0, src, mh0)
            s1 = pw.tile([128, HALF], MM_DT, tag="ks1")
            nc.vector.tensor_mul(s1, src, mh1)
            nc.scalar.dma_start(out=dst0, in_=s0)
            nc.scalar.dma_start(out=dst1, in_=s1)

        # ---- K projection (own tokens) + rope + staging + exchange ----
        for ug in range(2):
            accs = proj_group4(wk, ug)
            for jj in range(4):
                j = ug * 4 + jj
                t1, t2 = rope(accs[jj])
                rot = pw.tile([128, HALF], MM_DT, tag="rot")
                nc.vector.tensor_add(rot, t1, t2)
                stage_pair(rot, stageK[0, j], stageK[1, j])
        nc.gpsimd.collective_compute(
            "AllReduce", mybir.AluOpType.add, GROUPS,
            ins=[stageK], outs=[gathK])
        kT_s = kbuf.tile([128, KH, S], MM_DT, tag="k")

        # ---- V projection (transposed layout) + staging + exchange ----
        for vg in range(2):
            accs = projT_group4(wv, vg)
            for t4 in range(4):
                v_s = pw.tile([128, HALF], MM_DT, tag="qs")
                nc.scalar.copy(v_s, accs[t4])
                idx = t4 * 2 + vg
                stage_pair(v_s, stageV[0, idx], stageV[1, idx])
        nc.gpsimd.collective_compute(
            "AllReduce", mybir.AluOpType.add, GROUPS,
            ins=[stageV], outs=[gathV])
        v_g = vbuf.tile([128, 2, 8, HALF], MM_DT, tag="v")

        def v_ap(kvh, p):
            # [128 tok, 128 vd] tile of kv-head kvh at gathered position p
            return v_g[:, p // 4, (p % 4) * 2 + (kvh // 4),
                       (kvh % 4) * 128:(kvh % 4 + 1) * 128]

        # ---- Q projection + attention, interleaved per kv-group ----
        qT = qbuf.tile([128, H, HALF], MM_DT, tag="q")
        attnT = abuf.tile([128, H, HALF], MM_DT, tag="a")

        def attend(kvh, h):
            oT = ps_attn.tile([128, HALF], F32, tag="a2", name="oT")
            sum_ps = ps_attn.tile([128, HALF], F32, tag="a2", name="sum")
            for ti, t in enumerate(range(7, -1, -1)):
                cols = NCOLS[t]
                pos = TRAV[t]
                sc = ps_sc.tile([128, HALF], F32, tag="sc")
                nc.tensor.matmul(
                    sc[:, 0:cols], kT_s[:, kvh, pos * 128:(pos + 1) * 128],
                    qT[:, h, 0:cols], start=True, stop=True)
                p_t = pr.tile([128, HALF], MM_DT, tag="pr")
                nc.scalar.activation(p_t[:, 0:cols], sc[:, 0:cols],
                                     mybir.ActivationFunctionType.Exp,
                                     bias=m8, scale=SCALE)
                nc.vector.tensor_mul(p_t[:, cols - 128:cols],
                                     p_t[:, cols - 128:cols], dmask[:, t, :])
                nc.tensor.matmul(oT[:, 0:cols], v_ap(kvh, pos),
                                 p_t[:, 0:cols], start=(ti == 0),
                                 stop=(ti == 7))
                nc.tensor.matmul(sum_ps[:, 0:cols], ones, p_t[:, 0:cols],
                                 start=(ti == 0), stop=(ti == 7))
            rB2 = rb.tile([128, HALF], F32, tag="rb2")
            nc.vector.reciprocal_approx_fast(rB2, sum_ps)
            nc.vector.tensor_mul(attnT[:, h, :], oT, rB2)

        for g in range(8):
            accs = proj_group4(wq, g)
            for jj in range(4):
                t1, t2 = rope(accs[jj])
                nc.vector.tensor_add(qT[:, g * 4 + jj, :], t1, t2)
            if g == 3:
                # CC_K long done by the time the scalar queue reaches these
                for kvh in range(KH):
                    nc.scalar.dma_start(out=kT_s[:, kvh, 0:512],
                                        in_=gathK[0, kvh])
                    nc.scalar.dma_start(out=kT_s[:, kvh, 512:1024],
                                        in_=gathK[1, kvh])
            if g == 4:
                for half in range(2):
                    for idx in range(8):
                        nc.scalar.dma_start(out=v_g[:, half, idx, :],
                                            in_=gathV[half, idx])
            if g >= 5:
                for h in range(2 * (g - 5), 2 * (g - 5) + 2):
                    attend(h // 4, h)

        for h in range(6, 16):
            attend(h // 4, h)

        def out_djg(p, dst, djg):
            accs = [ps_proj.tile([128, HALF], F32, tag="acc",
                                 name=f"oacc{i}") for i in range(4)]
            for hq in range(4):
                wo_t = wbuf.tile([128, 4, HALF], MM_DT, tag="w")
                nc.sync.dma_start(
                    out=wo_t,
                    in_=wo[(p * 4 + hq) * 512:(p * 4 + hq + 1) * 512,
                           djg * 512:(djg + 1) * 512].rearrange(
                               "(t p) j -> p t j", p=128))
                for t in range(4):
                    hd = p * 16 + hq * 4 + t
                    for t4 in range(4):
                        nc.tensor.matmul(
                            accs[t4],
                            attnT[:, hd, t4 * 128:(t4 + 1) * 128],
                            wo_t[:, t, :],
                            start=(hq == 0 and t == 0),
                            stop=(hq == 3 and t == 3))
            for t4 in range(4):
                o_s = ow.tile([128, HALF], F32, tag="ow")
                if t4 % 2 == 0:
                    nc.scalar.copy(o_s, accs[t4])
                else:
                    nc.vector.tensor_copy(o_s, accs[t4])
                dq = nc.scalar if t4 % 2 == 0 else nc.sync
                dq.dma_start(
                    out=dst[t4 * 128:(t4 + 1) * 128,
                            djg * 512:(djg + 1) * 512],
                    in_=o_s)

        for djg in range(8):
            for h in (16 + 2 * djg, 17 + 2 * djg):
                attend(h // 4, h)
            out_djg(0, out0, djg)
        for djg in range(8):
            out_djg(1, out1, djg)

    nc.compile()
    return nc


def _get_compiled():
    global _compiled
    if _compiled is None:
        _compiled = _build()
    return _compiled


def _host_prep(x, freqs_cis, mask, wq, wk, wv, wo):
    """Shard + lay out inputs per core. Core 2b+h: batch b, q-row parity h."""
    del mask  # causal structure is hardcoded in the per-core mask tiles

    def ab_perm(n_heads):
        p = []
        for hh in range(n_heads):
            base = hh * HD
            p.extend(range(base, base + HD, 2))
            p.extend(range(base + 1, base + HD, 2))
        return np.asarray(p)

    wq_p = np.ascontiguousarray(np.asarray(wq)[:, ab_perm(H)]).astype(MM_NP)
    wk_p = np.ascontiguousarray(np.asarray(wk)[:, ab_perm(KH)]).astype(MM_NP)
    wv_p = np.asarray(wv).astype(MM_NP)
    wo_p = np.asarray(wo).astype(MM_NP)

    rotT = np.zeros((128, 128), dtype=MM_NP)
    rotT[np.arange(64), np.arange(64) + 64] = 1.0
    rotT[np.arange(64) + 64, np.arange(64)] = -1.0
    ones = np.ones((128, 128), dtype=MM_NP)

    x = np.asarray(x, dtype=np.float32)
    freqs = np.asarray(freqs_cis, dtype=np.float64)
    tri = (np.arange(128)[:, None] <= np.arange(128)[None, :])  # k_i <= q_j

    in_maps = []
    for c in range(N_CORES):
        b, h = divmod(c, 2)
        blocks = BLOCKS_A if h == 0 else BLOCKS_B
        perm = np.concatenate(
            [np.arange(r * 128, (r + 1) * 128) for r in blocks])
        xT_c = np.ascontiguousarray(x[b][perm].T).astype(MM_NP)
        f = freqs[perm]                        # [512, 64] angles
        cos_h = np.cos(f).T.astype(np.float32)  # [64, 512]
        sin_h = np.sin(f).T.astype(np.float32)
        cosT_c = np.ascontiguousarray(np.concatenate([cos_h, cos_h], axis=0))
        sinT_c = np.ascontiguousarray(np.concatenate([sin_h, sin_h], axis=0))
        # diag masks: A: odd t -> tri, even -> ones; B: even t -> tri, odd -> 0
        dmask_c = np.empty((128, 8, 128), dtype=MM_NP)
        for t in range(8):
            if h == 0:
                dmask_c[:, t, :] = tri if (t % 2 == 1) else 1.0
            else:
                dmask_c[:, t, :] = tri if (t % 2 == 0) else 0.0
        mh0_c = np.full((128, HALF), 1.0 - h, dtype=MM_NP)
        mh1_c = np.full((128, HALF), float(h), dtype=MM_NP)
        in_maps.append({
            "xT": xT_c, "wq": wq_p, "wk": wk_p, "wv": wv_p, "wo": wo_p,
            "cosT": cosT_c, "sinT": sinT_c, "dmask": dmask_c,
            "mh0": mh0_c, "mh1": mh1_c,
            "rotT": rotT, "ones": ones,
        })
    return in_maps


def kernel(x, freqs_cis, mask, wq, wk, wv, wo):
    nc = _get_compiled()
    in_maps = _host_prep(x, freqs_cis, mask, wq, wk, wv, wo)
    res = run_bass_kernel_spmd(nc, in_maps, list(range(N_CORES)))
    out = np.empty((B, S, D), dtype=np.float32)
    for c in range(N_CORES):
        b, h = divmod(c, 2)
        blocks = BLOCKS_A if h == 0 else BLOCKS_B
        perm = np.concatenate(
            [np.arange(r * 128, (r + 1) * 128) for r in blocks])
        out[b, perm, :] = res.results[c]["out0"] + res.results[c]["out1"]
    return out


# revision 14
# speedup vs baseline: 1.0056x; 1.0056x over previous
"""GQA causal attention block (B=4, S=1024, D=4096, H=32, KH=8, HD=128) on 8
Trainium2 NeuronCores.

Sharding v3: data-parallel over (batch, q-row-parity). Each batch b is split
between cores 2b (q-row blocks [7,5,3,1], "A") and 2b+1 (blocks [6,4,2,0],
"B"), 512 query tokens per core. Each core projects Q/K/V only for its OWN
512 tokens; the pair exchanges K and V via zero-padded pairwise AllReduces
(AllGather is broken in this runtime), so K/V projection work is not
duplicated. V is projected directly in transposed [tok, vdim] layout (x tile
as the stationary matmul operand), so no on-chip transposes are needed.

Causal load balance: q-row blocks are assigned so both cores of a pair need
the same per-slot key-prefix profile C=(8,6,4,2) (in 128-row k-tiles). The
attention loop iterates original k-tile index t=7..0 with a growing q-col
prefix (128,128,256,256,384,384,512,512 cols). The key tiles live in the
gathered buffer at fixed positions (traversal table); causality inside the
last 128-col block of each prefix is applied by multiplying the exp'd probs
with a per-core 0/1 mask tile (data), everything else uses a uniform -8 exp
bias. This computes 20/32 of the dense score tiles per head.

The output projection runs as two passes of 16 heads (the second overlaps
the attention tail); the two partial outputs are summed on the host.

RoPE trick (as baseline): wq/wk columns host-permuted per head to [even|odd]
halves; rot(q) = q*cos + (S@q)*sin with a constant 128x128 swap matrix.
cos/sin tables are precomputed on host from freqs_cis.

Matmuls in fp16; softmax statistics in fp32; exp biased by -8 (cancelled by
the normalization).
"""

import numpy as np

import concourse.bass as bass
import concourse.tile as tile
from concourse import bacc, mybir
from concourse.bass_utils import run_bass_kernel_spmd

B, S, D = 4, 1024, 4096
H, KH, HD = 32, 8, 128
HALF = S // 2                    # tokens per core
N_CORES = 8
SCALE = 1.0 / float(np.sqrt(HD))
EXP_BIAS = -8.0

MM_DT = mybir.dt.float16
MM_NP = np.float16
F32 = mybir.dt.float32

DT = D // 128                    # 32 d-tiles
GROUPS = [[0, 1], [2, 3], [4, 5], [6, 7]]

# q-row blocks per core type (slot order, need-descending)
BLOCKS_A = [7, 5, 3, 1]
BLOCKS_B = [6, 4, 2, 0]
# gathered position of original k-row t: A blocks at 0..3, B at 4..7
TRAV = [None] * 8
for _s, _r in enumerate(BLOCKS_A):
    TRAV[_r] = _s
for _s, _r in enumerate(BLOCKS_B):
    TRAV[_r] = 4 + _s
# q-col prefix width when processing original k-tile t
NCOLS = [512, 512, 384, 384, 256, 256, 128, 128]

_compiled = None


def _build():
    nc = bacc.Bacc("TRN2", target_bir_lowering=False, debug=False,
                   num_devices=N_CORES)

    xT = nc.dram_tensor("xT", [D, HALF], MM_DT, kind="ExternalInput").ap()
    wq = nc.dram_tensor("wq", [D, H * HD], MM_DT, kind="ExternalInput").ap()
    wk = nc.dram_tensor("wk", [D, KH * HD], MM_DT, kind="ExternalInput").ap()
    wv = nc.dram_tensor("wv", [D, KH * HD], MM_DT, kind="ExternalInput").ap()
    wo = nc.dram_tensor("wo", [H * HD, D], MM_DT, kind="ExternalInput").ap()
    cosT_d = nc.dram_tensor("cosT", [128, HALF], F32, kind="ExternalInput").ap()
    sinT_d = nc.dram_tensor("sinT", [128, HALF], F32, kind="ExternalInput").ap()
    dmask_d = nc.dram_tensor("dmask", [128, 8, 128], MM_DT, kind="ExternalInput").ap()
    mh0_d = nc.dram_tensor("mh0", [128, HALF], MM_DT, kind="ExternalInput").ap()
    mh1_d = nc.dram_tensor("mh1", [128, HALF], MM_DT, kind="ExternalInput").ap()
    rotT_d = nc.dram_tensor("rotT", [128, 128], MM_DT, kind="ExternalInput").ap()
    ones_d = nc.dram_tensor("ones", [128, 128], MM_DT, kind="ExternalInput").ap()
    out0 = nc.dram_tensor("out0", [HALF, D], F32, kind="ExternalOutput").ap()
    out1 = nc.dram_tensor("out1", [HALF, D], F32, kind="ExternalOutput").ap()

    # pair-exchange staging (K: [hd, tok] j-tiles; V: [tok, vd] tiles)
    stageK = nc.dram_tensor("stageK", [2, 8, 128, HALF], MM_DT).ap()
    gathK = nc.dram_tensor("gathK", [2, 8, 128, HALF], MM_DT).ap()
    stageV = nc.dram_tensor("stageV", [2, 8, 128, HALF], MM_DT).ap()
    gathV = nc.dram_tensor("gathV", [2, 8, 128, HALF], MM_DT).ap()

    from contextlib import ExitStack

    es = ExitStack()
    with tile.TileContext(nc) as tc, es:
        const = es.enter_context(tc.tile_pool(name="const", bufs=1))
        xbuf = es.enter_context(tc.tile_pool(name="xbuf", bufs=1))
        qbuf = es.enter_context(tc.tile_pool(name="qbuf", bufs=1))
        abuf = es.enter_context(tc.tile_pool(name="abuf", bufs=1))
        kbuf = es.enter_context(tc.tile_pool(name="kbuf", bufs=1))
        vbuf = es.enter_context(tc.tile_pool(name="vbuf", bufs=1))
        wbuf = es.enter_context(tc.tile_pool(name="wbuf", bufs=6))
        pw = es.enter_context(tc.tile_pool(name="pw", bufs=3))
        pr = es.enter_context(tc.tile_pool(name="pr", bufs=3))
        rb = es.enter_context(tc.tile_pool(name="rb", bufs=2))
        ow = es.enter_context(tc.tile_pool(name="ow", bufs=3))
        ps_proj = es.enter_context(tc.tile_pool(name="ps_proj", bufs=4, space="PSUM"))
        ps_attn = es.enter_context(tc.tile_pool(name="ps_attn", bufs=2, space="PSUM"))
        ps_sc = es.enter_context(tc.tile_pool(name="ps_sc", bufs=2, space="PSUM"))

        # ---- x first (scalar queue; weights go on sync in parallel) ----
        xT_s = xbuf.tile([128, DT, HALF], MM_DT, tag="x")

        def load_x(dc):
            nc.scalar.dma_start(
                out=xT_s[:, dc * 4:(dc + 1) * 4, :],
                in_=xT[dc * 512:(dc + 1) * 512, :].rearrange(
                    "(t p) c -> p t c", p=128))

        nc.scalar.dma_start(
            out=xT_s[:, 0:2, :],
            in_=xT[0:256, :].rearrange("(t p) c -> p t c", p=128))
        nc.scalar.dma_start(
            out=xT_s[:, 2:4, :],
            in_=xT[256:512, :].rearrange("(t p) c -> p t c", p=128))
        load_x(1)

        # ---- constants (scalar queue) ----
        rotT = const.tile([128, 128], MM_DT, tag="rot")
        nc.scalar.dma_start(out=rotT, in_=rotT_d)
        ones = const.tile([128, 128], MM_DT, tag="ones")
        nc.scalar.dma_start(out=ones, in_=ones_d)
        cosT = const.tile([128, HALF], F32, tag="cos")
        nc.scalar.dma_start(out=cosT, in_=cosT_d)
        sinT = const.tile([128, HALF], F32, tag="sin")
        nc.scalar.dma_start(out=sinT, in_=sinT_d)
        dmask = const.tile([128, 8, 128], MM_DT, tag="dmask")
        nc.scalar.dma_start(out=dmask, in_=dmask_d)
        mh0 = const.tile([128, HALF], MM_DT, tag="mh0")
        nc.scalar.dma_start(out=mh0, in_=mh0_d)
        mh1 = const.tile([128, HALF], MM_DT, tag="mh1")
        nc.scalar.dma_start(out=mh1, in_=mh1_d)
        m8 = const.tile([128, 1], F32, tag="m8")
        nc.vector.memset(m8, EXP_BIAS)
        for dc in range(2, 8):
            load_x(dc)

        def proj_group4(w_ap, jg):
            """Project 4 j-tiles (cols jg*512..+512 of w_ap) over all of D."""
            accs = [ps_proj.tile([128, HALF], F32, tag="acc", name=f"acc{i}")
                    for i in range(4)]
            for dp in range(8):
                w_t = wbuf.tile([128, 4, HALF], MM_DT, tag="w")
                nc.sync.dma_start(
                    out=w_t,
                    in_=w_ap[dp * 512:(dp + 1) * 512,
                             jg * 512:(jg + 1) * 512].rearrange(
                                 "(t p) j -> p t j", p=128))
                for t in range(4):
                    for jj in range(4):
                        nc.tensor.matmul(
                            accs[jj], w_t[:, t, jj * 128:(jj + 1) * 128],
                            xT_s[:, dp * 4 + t, :],
                            start=(dp == 0 and t == 0),
                            stop=(dp == 7 and t == 3))
            return accs

        def projT_group4(w_ap, vg):
            """Project 4 token-tiles of (x @ w[:, vg*512:+512]) TRANSPOSED:
            accs[t4] = [128 tok, 512 vdim], x tile as stationary operand."""
            accs = [ps_proj.tile([128, HALF], F32, tag="acc", name=f"vacc{i}")
                    for i in range(4)]
            for dp in range(8):
                w_t = wbuf.tile([128, 4, HALF], MM_DT, tag="w")
                nc.sync.dma_start(
                    out=w_t,
                    in_=w_ap[dp * 512:(dp + 1) * 512,
                             vg * 512:(vg + 1) * 512].rearrange(
                                 "(t p) j -> p t j", p=128))
                for t in range(4):
                    for t4 in range(4):
                        nc.tensor.matmul(
                            accs[t4],
                            xT_s[:, dp * 4 + t, t4 * 128:(t4 + 1) * 128],
                            w_t[:, t, :],
                            start=(dp == 0 and t == 0),
                            stop=(dp == 7 and t == 3))
            return accs

        def rope(acc):
            q_s = pw.tile([128, HALF], MM_DT, tag="qs")
            nc.scalar.copy(q_s, acc)
            ps2 = ps_proj.tile([128, HALF], F32, tag="acc", name="rot")
            nc.tensor.matmul(ps2, rotT, q_s, start=True, stop=True)
            t1 = pw.tile([128, HALF], F32, tag="t1")
            nc.vector.tensor_mul(t1, q_s, cosT)
            t2 = pw.tile([128, HALF], F32, tag="t2")
            nc.vector.tensor_mul(t2, ps2, sinT)
            return t1, t2

        def stage_pair(src, dst0, dst1):
            s0 = pw.tile([128, HALF], MM_DT, tag="ks0")
            nc.vector.tensor_mul(s0, src, mh0)
            s1 = pw.tile([128, HALF], MM_DT, tag="ks1")
            nc.vector.tensor_mul(s1, src, mh1)
            nc.scalar.dma_start(out=dst0, in_=s0)
            nc.scalar.dma_start(out=dst1, in_=s1)

        # ---- K projection (own tokens) + rope + staging + exchange ----
        for ug in range(2):
            accs = proj_group4(wk, ug)
            for jj in range(4):
                j = ug * 4 + jj
                t1, t2 = rope(accs[jj])
                rot = pw.tile([128, HALF], MM_DT, tag="rot")
                nc.vector.tensor_add(rot, t1, t2)
                stage_pair(rot, stageK[0, j], stageK[1, j])
        nc.gpsimd.collective_compute(
            "AllReduce", mybir.AluOpType.add, GROUPS,
            ins=[stageK], outs=[gathK])
        kT_s = kbuf.tile([128, KH, S], MM_DT, tag="k")

        # ---- V projection (transposed layout) + staging + exchange ----
        for vg in range(2):
            accs = projT_group4(wv, vg)
            for t4 in range(4):
                v_s = pw.tile([128, HALF], MM_DT, tag="qs")
                nc.scalar.copy(v_s, accs[t4])
                idx = t4 * 2 + vg
                stage_pair(v_s, stageV[0, idx], stageV[1, idx])
        nc.gpsimd.collective_compute(
            "AllReduce", mybir.AluOpType.add, GROUPS,
            ins=[stageV], outs=[gathV])
        v_g = vbuf.tile([128, 2, 8, HALF], MM_DT, tag="v")

        def v_ap(kvh, p):
            # [128 tok, 128 vd] tile of kv-head kvh at gathered position p
            return v_g[:, p // 4, (p % 4) * 2 + (kvh // 4),
                       (kvh % 4) * 128:(kvh % 4 + 1) * 128]

        # ---- Q projection + attention, interleaved per kv-group ----
        qT = qbuf.tile([128, H, HALF], MM_DT, tag="q")
        attnT = abuf.tile([128, H, HALF], MM_DT, tag="a")

        def attend(kvh, h):
            oT = ps_attn.tile([128, HALF], F32, tag="a2", name="oT")
            sum_ps = ps_attn.tile([128, HALF], F32, tag="a2", name="sum")
            for ti, t in enumerate(range(7, -1, -1)):
                cols = NCOLS[t]
                pos = TRAV[t]
                sc = ps_sc.tile([128, HALF], F32, tag="sc")
                nc.tensor.matmul(
                    sc[:, 0:cols], kT_s[:, kvh, pos * 128:(pos + 1) * 128],
                    qT[:, h, 0:cols], start=True, stop=True)
                p_t = pr.tile([128, HALF], MM_DT, tag="pr")
                nc.scalar.activation(p_t[:, 0:cols], sc[:, 0:cols],
                                     mybir.ActivationFunctionType.Exp,
                                     bias=m8, scale=SCALE)
                nc.vector.tensor_mul(p_t[:, cols - 128:cols],
                                     p_t[:, cols - 128:cols], dmask[:, t, :])
                nc.tensor.matmul(oT[:, 0:cols], v_ap(kvh, pos),
                                 p_t[:, 0:cols], start=(ti == 0),
                                 stop=(ti == 7))
                nc.tensor.matmul(sum_ps[:, 0:cols], ones, p_t[:, 0:cols],
                                 start=(ti == 0), stop=(ti == 7))
            rB2 = rb.tile([128, HALF], F32, tag="rb2")
            nc.vector.reciprocal_approx_fast(rB2, sum_ps)
            nc.vector.tensor_mul(attnT[:, h, :], oT, rB2)

        for g in range(8):
            accs = proj_group4(wq, g)
            for jj in range(4):
                t1, t2 = rope(accs[jj])
                nc.vector.tensor_add(qT[:, g * 4 + jj, :], t1, t2)
            if g == 3:
                # CC_K long done by the time the scalar queue reaches these
                for kvh in range(KH):
                    nc.scalar.dma_start(out=kT_s[:, kvh, 0:512],
                                        in_=gathK[0, kvh])
                    nc.scalar.dma_start(out=kT_s[:, kvh, 512:1024],
                                        in_=gathK[1, kvh])
            if g == 4:
                for half in range(2):
                    for idx in range(8):
                        nc.scalar.dma_start(out=v_g[:, half, idx, :],
                                            in_=gathV[half, idx])
            if g >= 5:
                for h in range(2 * (g - 5), 2 * (g - 5) + 2):
                    attend(h // 4, h)

        for h in range(6, 16):
            attend(h // 4, h)

        def out_djg(p, dst, djg):
            accs = [ps_proj.tile([128, HALF], F32, tag="acc",
                                 name=f"oacc{i}") for i in range(4)]
            for hq in range(4):
                wo_t = wbuf.tile([128, 4, HALF], MM_DT, tag="w")
                nc.sync.dma_start(
                    out=wo_t,
                    in_=wo[(p * 4 + hq) * 512:(p * 4 + hq + 1) * 512,
                           djg * 512:(djg + 1) * 512].rearrange(
                               "(t p) j -> p t j", p=128))
                for t in range(4):
                    hd = p * 16 + hq * 4 + t
                    for t4 in range(4):
                        nc.tensor.matmul(
                            accs[t4],
                            attnT[:, hd, t4 * 128:(t4 + 1) * 128],
                            wo_t[:, t, :],
                            start=(hq == 0 and t == 0),
                            stop=(hq == 3 and t == 3))
            for t4 in range(4):
                o_s = ow.tile([128, HALF], F32, tag="ow")
                if t4 % 2 == 0:
                    nc.scalar.copy(o_s, accs[t4])
                else:
                    nc.vector.tensor_copy(o_s, accs[t4])
                dq = nc.scalar if t4 % 2 == 0 else nc.sync
                dq.dma_start(
                    out=dst[t4 * 128:(t4 + 1) * 128,
                            djg * 512:(djg + 1) * 512],
                    in_=o_s)

        for djg in range(8):
            for h in (16 + 2 * djg, 17 + 2 * djg):
                attend(h // 4, h)
            out_djg(0, out0, djg)
        for djg in range(8):
            out_djg(1, out1, djg)

    nc.compile()
    return nc


def _get_compiled():
    global _compiled
    if _compiled is None:
        _compiled = _build()
    return _compiled


def _host_prep(x, freqs_cis, mask, wq, wk, wv, wo):
    """Shard + lay out inputs per core. Core 2b+h: batch b, q-row parity h."""
    del mask  # causal structure is hardcoded in the per-core mask tiles

    def ab_perm(n_heads):
        p = []
        for hh in range(n_heads):
            base = hh * HD
            p.extend(range(base, base + HD, 2))
            p.extend(range(base + 1, base + HD, 2))
        return np.asarray(p)

    wq_p = np.ascontiguousarray(np.asarray(wq)[:, ab_perm(H)]).astype(MM_NP)
    wk_p = np.ascontiguousarray(np.asarray(wk)[:, ab_perm(KH)]).astype(MM_NP)
    wv_p = np.asarray(wv).astype(MM_NP)
    wo_p = np.asarray(wo).astype(MM_NP)

    rotT = np.zeros((128, 128), dtype=MM_NP)
    rotT[np.arange(64), np.arange(64) + 64] = 1.0
    rotT[np.arange(64) + 64, np.arange(64)] = -1.0
    ones = np.ones((128, 128), dtype=MM_NP)

    x = np.asarray(x, dtype=np.float32)
    freqs = np.asarray(freqs_cis, dtype=np.float64)
    tri = (np.arange(128)[:, None] <= np.arange(128)[None, :])  # k_i <= q_j

    in_maps = []
    for c in range(N_CORES):
        b, h = divmod(c, 2)
        blocks = BLOCKS_A if h == 0 else BLOCKS_B
        perm = np.concatenate(
            [np.arange(r * 128, (r + 1) * 128) for r in blocks])
        xT_c = np.ascontiguousarray(x[b][perm].T).astype(MM_NP)
        f = freqs[perm]                        # [512, 64] angles
        cos_h = np.cos(f).T.astype(np.float32)  # [64, 512]
        sin_h = np.sin(f).T.astype(np.float32)
        cosT_c = np.ascontiguousarray(np.concatenate([cos_h, cos_h], axis=0))
        sinT_c = np.ascontiguousarray(np.concatenate([sin_h, sin_h], axis=0))
        # diag masks: A: odd t -> tri, even -> ones; B: even t -> tri, odd -> 0
        dmask_c = np.empty((128, 8, 128), dtype=MM_NP)
        for t in range(8):
            if h == 0:
                dmask_c[:, t, :] = tri if (t % 2 == 1) else 1.0
            else:
                dmask_c[:, t, :] = tri if (t % 2 == 0) else 0.0
        mh0_c = np.full((128, HALF), 1.0 - h, dtype=MM_NP)
        mh1_c = np.full((128, HALF), float(h), dtype=MM_NP)
        in_maps.append({
            "xT": xT_c, "wq": wq_p, "wk": wk_p, "wv": wv_p, "wo": wo_p,
            "cosT": cosT_c, "sinT": sinT_c, "dmask": dmask_c,
            "mh0": mh0_c, "mh1": mh1_c,
            "rotT": rotT, "ones": ones,
        })
    return in_maps


def kernel(x, freqs_cis, mask, wq, wk, wv, wo):
    nc = _get_compiled()
    in_maps = _host_prep(x, freqs_cis, mask, wq, wk, wv, wo)
    res = run_bass_kernel_spmd(nc, in_maps, list(range(N_CORES)))
    out = np.empty((B, S, D), dtype=np.float32)
    for c in range(N_CORES):
        b, h = divmod(c, 2)
        blocks = BLOCKS_A if h == 0 else BLOCKS_B
        perm = np.concatenate(
            [np.arange(r * 128, (r + 1) * 128) for r in blocks])
        out[b, perm, :] = res.results[c]["out0"] + res.results[c]["out1"]
    return out
